# revision 6
# baseline (speedup 1.0000x reference)
"""Trainium2 Bass kernel for nn_DSA_Equal (dual self/cross attention block).

Sharding: 8 cores = (batch b in {0,1}) x (query-quarter q in {0..3}).
Each core computes, for its 1024 query positions of batch b, BOTH
attention branches (self + cross): the two [1024, 4096] score slabs,
and the y = relu(Wout @ [target_s; target_c] + b) slice plus BN partial
stats. A second tiny launch applies batchnorm with the globally-reduced
stats. Host code only does input prep (masking/transposes/weight
folding) and output assembly.

Math notes:
 - softmax without max-subtraction (scores are O(1) here; exp is safe).
 - scoresT ([m, n] layout) computed for the PV matmul; scores ([n, m])
   recomputed via a second matmul orientation for the output slab.
 - target/se_feat/Wout are folded on the host:
     y = (Wout_s@W_s) feat_s + (Wout_c@W_c) feat_c + (Wout_s+Wout_c) x + b
 - matmul operands fp16 (1 cycle/row on PE), accumulation fp32 in PSUM.
 - K=64 contractions are packed two-at-a-time onto the 128-row PE array
   via tile_position row/col groups (Q/K duplicated across partitions).
"""

import numpy as np

import concourse.bacc as bacc
import concourse.mybir as mybir
from concourse.tile import TileContext
from concourse.bass_utils import run_bass_kernel_spmd

F32 = mybir.dt.float32
F16 = mybir.dt.float16
AF = mybir.ActivationFunctionType

NCORES = 8
B, C, I, H, W = 2, 64, 64, 64, 64
N = H * W            # 4096
NSL = N // 4         # 1024 query positions per core
BN_EPS = 1e-5
NT = 128             # partition tile
MCH = 512            # matmul free-dim chunk

_CACHE = {}


def _build_main():
    nc = bacc.Bacc("TRN2", target_bir_lowering=False, debug=False,
                   num_devices=NCORES)

    # ---- per-core inputs (host pre-masked / transposed / folded) ----
    f_in = nc.dram_tensor("f_in", [C, N], F16, kind="ExternalInput")
    g_in = nc.dram_tensor("g_in", [C, N], F16, kind="ExternalInput")
    fq_in = nc.dram_tensor("fq_in", [C, NSL], F16, kind="ExternalInput")
    xq_in = nc.dram_tensor("xq_in", [C, NSL], F16, kind="ExternalInput")
    wname = ["thT_s", "phT_s", "gaT_s", "thT_c", "phT_c", "gaT_c",
             "AsT", "AcT", "XwT"]
    wt = {n: nc.dram_tensor(n, [64, 64], F16, kind="ExternalInput")
          for n in wname}
    ones1x64 = nc.dram_tensor("ones1x64", [1, 64], F16, kind="ExternalInput")
    wb = nc.dram_tensor("wb", [C, 1], F32, kind="ExternalInput")

    # ---- per-core outputs ----
    sco_s = nc.dram_tensor("sco_s", [NSL, N], F32, kind="ExternalOutput")
    sco_c = nc.dram_tensor("sco_c", [NSL, N], F32, kind="ExternalOutput")
    y_out = nc.dram_tensor("y_out", [C, NSL], F32, kind="ExternalOutput")
    st_out = nc.dram_tensor("st_out", [C, 2], F32, kind="ExternalOutput")

    # dram scratch for the [128, 8] -> [1, 1024] transpose bounce
    rb_d = {"s": nc.dram_tensor("rb_s", [1, NT, 8], F32),
            "c": nc.dram_tensor("rb_c", [1, NT, 8], F32)}

    with TileContext(nc) as tc:
        with (
            tc.tile_pool(name="const", bufs=1) as cpool,
            tc.tile_pool(name="fg", bufs=1) as fg,
            tc.tile_pool(name="proj", bufs=2) as proj,
            tc.tile_pool(name="epool", bufs=4) as epool,
            tc.tile_pool(name="stage", bufs=2) as stpool,
            tc.tile_pool(name="small", bufs=2) as small,
            tc.tile_pool(name="ps_a", bufs=2, space="PSUM") as ps_a,
            tc.tile_pool(name="ps_b", bufs=2, space="PSUM") as ps_b,
            tc.tile_pool(name="ps_ft", bufs=1, space="PSUM") as ps_ft,
            tc.tile_pool(name="ps_epi", bufs=1, space="PSUM") as ps_epi,
        ):
            # ---------- load constants / inputs ----------
            wsb = {}
            for n in wname:
                t = cpool.tile([64, 64], F16, name=f"w_{n}")
                nc.sync.dma_start(out=t[:, :], in_=wt[n][:, :])
                wsb[n] = t
            ones_sb = cpool.tile([1, 64], F16)
            nc.sync.dma_start(out=ones_sb[:, :], in_=ones1x64[:, :])
            wb_sb = cpool.tile([C, 1], F32)
            nc.sync.dma_start(out=wb_sb[:, :], in_=wb[:, :])

            f_sb = fg.tile([C, N], F16)
            g_sb = fg.tile([C, N], F16)
            nc.sync.dma_start(out=f_sb[:, :], in_=f_in[:, :])
            nc.sync.dma_start(out=g_sb[:, :], in_=g_in[:, :])
            fq_sb = fg.tile([C, NSL], F16)
            xq_sb = fg.tile([C, NSL], F16)
            nc.sync.dma_start(out=fq_sb[:, :], in_=fq_in[:, :])
            nc.sync.dma_start(out=xq_sb[:, :], in_=xq_in[:, :])

            feat16 = {}   # branch -> [64, NSL] f16 normalized feat

            for br, kin_sb in (("s", f_sb), ("c", g_sb)):
                thT, phT, gaT = wsb[f"thT_{br}"], wsb[f"phT_{br}"], wsb[f"gaT_{br}"]

                # ---------- projections ----------
                # Qd [128, NSL] f16: rows 0-63 and 64-127 both hold Q
                qd_sb = proj.tile([2 * 64, NSL], F16, name=f"qd_{br}", tag="qd")
                for nch in range(NSL // MCH):
                    cs = slice(nch * MCH, (nch + 1) * MCH)
                    qp = ps_a.tile([128, MCH], F32, tag="psA")
                    nc.tensor.matmul(qp[0:64, :], phT[:, :], fq_sb[:, cs],
                                     start=True, stop=True)
                    nc.tensor.matmul(qp[64:128, :], phT[:, :], fq_sb[:, cs],
                                     start=True, stop=True,
                                     tile_position=(0, 64),
                                     skip_group_check=True)
                    nc.vector.tensor_copy(qd_sb[:, cs], qp[:, :])

                # Kd [128, N] f16 (duplicated rows)
                kd_sb = proj.tile([2 * 64, N], F16, name=f"kd_{br}", tag="kd")
                for mch in range(N // MCH):
                    cs = slice(mch * MCH, (mch + 1) * MCH)
                    kp = ps_a.tile([128, MCH], F32, tag="psA")
                    nc.tensor.matmul(kp[0:64, :], thT[:, :], kin_sb[:, cs],
                                     start=True, stop=True)
                    nc.tensor.matmul(kp[64:128, :], thT[:, :], kin_sb[:, cs],
                                     start=True, stop=True,
                                     tile_position=(0, 64),
                                     skip_group_check=True)
                    nc.vector.tensor_copy(kd_sb[:, cs], kp[:, :])

                # Vt [128, 32*64] f16: Vt[:, t*64:(t+1)*64] = V^T tile t
                vt_sb = proj.tile([NT, (N // NT) * 64], F16,
                                  name=f"vt_{br}", tag="vt")
                for grp in range(N // NT // 8):   # 8 V^T tiles per psum bank
                    vp = ps_b.tile([128, 8 * 64], F32, tag="psB")
                    for k in range(8):
                        t = grp * 8 + k
                        nc.tensor.matmul(
                            vp[:, k * 64:(k + 1) * 64],
                            kin_sb[:, t * NT:(t + 1) * NT],
                            gaT[:, :], start=True, stop=True,
                            skip_group_check=True)
                    nc.vector.tensor_copy(
                        vt_sb[:, grp * 8 * 64:(grp + 1) * 8 * 64], vp[:, :])

                # ---------- Loop1: S^T -> exp -> PV accumulate ----------
                ftu = ps_ft.tile([128, NSL], F32, name=f"ftu_{br}", tag="ftu")
                for t in range(N // NT):          # 32 m-tiles
                    rows = slice(0, 64) if t % 2 == 0 else slice(64, 128)
                    e_sb = epool.tile([NT, NSL], F16, tag="etile")
                    for nch in range(NSL // MCH):
                        cs = slice(nch * MCH, (nch + 1) * MCH)
                        stp = ps_b.tile([128, MCH], F32, tag="psB")
                        nc.tensor.matmul(
                            stp[:, :], kd_sb[rows, t * NT:(t + 1) * NT],
                            qd_sb[rows, cs],
                            start=True, stop=True, skip_group_check=True)
                        nc.scalar.activation(e_sb[:, cs], stp[:, :], AF.Exp)
                    fparts = slice(0, 64) if t % 2 == 0 else slice(64, 128)
                    tpos = (0, 0) if t % 2 == 0 else (0, 64)
                    for nch in range(NSL // MCH):
                        cs = slice(nch * MCH, (nch + 1) * MCH)
                        nc.tensor.matmul(
                            ftu[fparts, cs], vt_sb[:, t * 64:(t + 1) * 64],
                            e_sb[:, cs],
                            start=(t < 2), stop=(t >= N // NT - 2),
                            tile_position=tpos, skip_group_check=True)

                # combine column-packed halves -> featU [64, NSL] f32
                # (DVE may read only one PSUM operand: bounce one half)
                ftu_hi = small.tile([64, NSL], F32, tag="ftu_hi")
                nc.vector.tensor_copy(ftu_hi[:, :], ftu[64:128, :])
                ftu_sb = small.tile([64, NSL], F32, tag="ftu_sb")
                nc.vector.tensor_add(ftu_sb[:, :], ftu[0:64, :], ftu_hi[:, :])

                # ---------- Loop2: S -> exp(+accum) -> normalize -> out ----
                rT_sb = small.tile([NT, 8], F32, tag="rT")
                sco = sco_s if br == "s" else sco_c
                for j in range(NSL // NT):        # 8 n-tiles (row-pack pairs)
                    rows = slice(0, 64) if j % 2 == 0 else slice(64, 128)
                    stg = stpool.tile([NT, N], F32, tag="stg")
                    acc8 = small.tile([NT, 8], F32, tag="acc8")
                    for mch in range(N // MCH):
                        cs = slice(mch * MCH, (mch + 1) * MCH)
                        sp = ps_a.tile([128, MCH], F32, tag="psA")
                        nc.tensor.matmul(
                            sp[:, :], qd_sb[rows, j * NT:(j + 1) * NT],
                            kd_sb[rows, cs],
                            start=True, stop=True, skip_group_check=True)
                        nc.scalar.activation(stg[:, cs], sp[:, :], AF.Exp,
                                             accum_out=acc8[:, mch:mch + 1])
                    ssum = small.tile([NT, 1], F32, tag="ssum")
                    nc.vector.reduce_sum(ssum[:, :], acc8[:, :],
                                         axis=mybir.AxisListType.X)
                    nc.vector.reciprocal(rT_sb[:, j:j + 1], ssum[:, :])
                    stgn = stpool.tile([NT, N], F32, tag="stgn")
                    nc.vector.tensor_scalar_mul(stgn[:, :], stg[:, :],
                                                rT_sb[:, j:j + 1])
                    nc.sync.dma_start(out=sco[j * NT:(j + 1) * NT, :],
                                      in_=stgn[:, :])

                # ---------- feat normalization ----------
                nc.sync.dma_start(out=rb_d[br][0, :, :], in_=rT_sb[:, :])
                rf_sb = small.tile([1, NSL], F32, tag="rf")
                nc.sync.dma_start(
                    out=rf_sb[:, :],
                    in_=rb_d[br].ap().rearrange("a p j -> a j p"))
                rf16_sb = small.tile([1, NSL], F16, tag="rf16")
                nc.vector.tensor_copy(rf16_sb[:, :], rf_sb[:, :])

                rbc = ps_epi.tile([64, NSL], F32, name=f"rbc_{br}", tag="rbc")
                for nch in range(NSL // MCH):
                    cs = slice(nch * MCH, (nch + 1) * MCH)
                    nc.tensor.matmul(rbc[:, cs], ones_sb[:, :], rf16_sb[:, cs],
                                     start=True, stop=True,
                                     skip_group_check=True)
                f16t = small.tile([64, NSL], F16, tag="feat16")
                nc.vector.tensor_mul(f16t[:, :], ftu_sb[:, :], rbc[:, :])
                feat16[br] = f16t

            # ---------- epilogue: y, relu, stats ----------
            yps = ps_epi.tile([64, NSL], F32, name="yps", tag="rbc")
            for nch in range(NSL // MCH):
                cs = slice(nch * MCH, (nch + 1) * MCH)
                nc.tensor.matmul(yps[:, cs], wsb["AsT"][:, :],
                                 feat16["s"][:, cs],
                                 start=True, stop=False, skip_group_check=True)
                nc.tensor.matmul(yps[:, cs], wsb["AcT"][:, :],
                                 feat16["c"][:, cs],
                                 start=False, stop=False, skip_group_check=True)
                nc.tensor.matmul(yps[:, cs], wsb["XwT"][:, :], xq_sb[:, cs],
                                 start=False, stop=True, skip_group_check=True)
            y_sb = small.tile([C, NSL], F32, tag="ysb")
            nc.scalar.activation(y_sb[:, :], yps[:, :], AF.Relu,
                                 bias=wb_sb[:, :])
            nc.sync.dma_start(out=y_out[:, :], in_=y_sb[:, :])

            st_sb = small.tile([C, 2], F32, tag="stats")
            nc.vector.reduce_sum(st_sb[:, 0:1], y_sb[:, :],
                                 axis=mybir.AxisListType.X)
            y2_sb = small.tile([C, NSL], F32, tag="y2")
            nc.vector.tensor_mul(y2_sb[:, :], y_sb[:, :], y_sb[:, :])
            nc.vector.reduce_sum(st_sb[:, 1:2], y2_sb[:, :],
                                 axis=mybir.AxisListType.X)
            nc.sync.dma_start(out=st_out[:, :], in_=st_sb[:, :])

    nc.compile()
    return nc


def _build_bn():
    nc = bacc.Bacc("TRN2", target_bir_lowering=False, debug=False,
                   num_devices=NCORES)
    y_in = nc.dram_tensor("y_in", [C, NSL], F32, kind="ExternalInput")
    allst = nc.dram_tensor("allst", [C, 2 * NCORES], F32, kind="ExternalInput")
    gam = nc.dram_tensor("gam", [C, 1], F32, kind="ExternalInput")
    bet = nc.dram_tensor("bet", [C, 1], F32, kind="ExternalInput")
    o_out = nc.dram_tensor("o_out", [C, NSL], F32, kind="ExternalOutput")

    inv_n = 1.0 / (B * N)
    with TileContext(nc) as tc:
        with tc.tile_pool(name="sb", bufs=1) as sb:
            y_sb = sb.tile([C, NSL], F32)
            st_sb = sb.tile([C, 2 * NCORES], F32)
            g_sb = sb.tile([C, 1], F32)
            b_sb = sb.tile([C, 1], F32)
            nc.sync.dma_start(out=y_sb[:, :], in_=y_in[:, :])
            nc.sync.dma_start(out=st_sb[:, :], in_=allst[:, :])
            nc.sync.dma_start(out=g_sb[:, :], in_=gam[:, :])
            nc.sync.dma_start(out=b_sb[:, :], in_=bet[:, :])

            s1 = sb.tile([C, 1], F32)
            s2 = sb.tile([C, 1], F32)
            nc.vector.reduce_sum(s1[:, :], st_sb[:, 0:NCORES],
                                 axis=mybir.AxisListType.X)
            nc.vector.reduce_sum(s2[:, :], st_sb[:, NCORES:2 * NCORES],
                                 axis=mybir.AxisListType.X)
            mean = sb.tile([C, 1], F32)
            ex2 = sb.tile([C, 1], F32)
            nc.vector.tensor_scalar_mul(mean[:, :], s1[:, :], inv_n)
            nc.vector.tensor_scalar_mul(ex2[:, :], s2[:, :], inv_n)
            m2 = sb.tile([C, 1], F32)
            nc.vector.tensor_mul(m2[:, :], mean[:, :], mean[:, :])
            var = sb.tile([C, 1], F32)
            nc.vector.tensor_sub(var[:, :], ex2[:, :], m2[:, :])
            vpe = sb.tile([C, 1], F32)
            nc.vector.tensor_scalar_add(vpe[:, :], var[:, :], BN_EPS)
            rv = sb.tile([C, 1], F32)
            nc.vector.reciprocal(rv[:, :], vpe[:, :])
            rstd = sb.tile([C, 1], F32)
            nc.scalar.activation(rstd[:, :], rv[:, :], AF.Sqrt)
            scale = sb.tile([C, 1], F32)
            nc.vector.tensor_mul(scale[:, :], g_sb[:, :], rstd[:, :])
            ms = sb.tile([C, 1], F32)
            nc.vector.tensor_mul(ms[:, :], mean[:, :], scale[:, :])
            shift = sb.tile([C, 1], F32)
            nc.vector.tensor_sub(shift[:, :], b_sb[:, :], ms[:, :])
            o_sb = sb.tile([C, NSL], F32)
            nc.vector.tensor_scalar(o_sb[:, :], y_sb[:, :], scale[:, :],
                                    shift[:, :], op0=mybir.AluOpType.mult,
                                    op1=mybir.AluOpType.add)
            nc.sync.dma_start(out=o_out[:, :], in_=o_sb[:, :])
    nc.compile()
    return nc


def _get_kernels():
    if "main" not in _CACHE:
        _CACHE["main"] = _build_main()
        _CACHE["bn"] = _build_bn()
    return _CACHE["main"], _CACHE["bn"]


def _prep_inputs(x, mask, theta_s, phi_s, gate_s, W_s, theta_c, phi_c, gate_c,
                 W_c, Wout_w, Wout_b):
    """Build the 8 per-core input maps for the main kernel."""
    f16 = np.float16
    x = np.asarray(x, np.float32)
    mask = np.asarray(mask, np.float32)

    A_s = np.asarray(Wout_w[:, :I], np.float32) @ np.asarray(W_s, np.float32)
    A_c = np.asarray(Wout_w[:, I:], np.float32) @ np.asarray(W_c, np.float32)
    Xw = (np.asarray(Wout_w[:, :I], np.float32)
          + np.asarray(Wout_w[:, I:], np.float32))

    const = {
        "thT_s": np.ascontiguousarray(np.asarray(theta_s).T).astype(f16),
        "phT_s": np.ascontiguousarray(np.asarray(phi_s).T).astype(f16),
        "gaT_s": np.ascontiguousarray(np.asarray(gate_s).T).astype(f16),
        "thT_c": np.ascontiguousarray(np.asarray(theta_c).T).astype(f16),
        "phT_c": np.ascontiguousarray(np.asarray(phi_c).T).astype(f16),
        "gaT_c": np.ascontiguousarray(np.asarray(gate_c).T).astype(f16),
        "AsT": np.ascontiguousarray(A_s.T).astype(f16),
        "AcT": np.ascontiguousarray(A_c.T).astype(f16),
        "XwT": np.ascontiguousarray(Xw.T).astype(f16),
        "ones1x64": np.ones((1, 64), f16),
        "wb": np.asarray(Wout_b, np.float32).reshape(C, 1),
    }

    in_maps = []
    for core in range(NCORES):
        b, q = divmod(core, 4)
        xb = x[b].reshape(C, N)
        mk = mask[b].reshape(1, N)
        fb = (xb * mk).astype(f16)
        gb = (xb * (1.0 - mk)).astype(f16)
        sl = slice(q * NSL, (q + 1) * NSL)
        m = {
            "f_in": fb,
            "g_in": gb,
            "fq_in": np.ascontiguousarray(fb[:, sl]),
            "xq_in": xb[:, sl].astype(f16),
        }
        m.update(const)
        in_maps.append(m)
    return in_maps


def kernel(x, mask, theta_s, phi_s, gate_s, W_s, theta_c, phi_c, gate_c,
           W_c, Wout_w, Wout_b, bn_gamma, bn_beta):
    nc_main, nc_bn = _get_kernels()

    in_maps = _prep_inputs(x, mask, theta_s, phi_s, gate_s, W_s,
                           theta_c, phi_c, gate_c, W_c, Wout_w, Wout_b)
    res = run_bass_kernel_spmd(nc_main, in_maps, core_ids=list(range(NCORES)))
    r = res.results

    # stats layout for the BN kernel: [C, 16] = [s1 of 8 cores | s2 of 8]
    allst = np.concatenate(
        [np.stack([r[c]["st_out"][:, 0] for c in range(NCORES)], axis=1),
         np.stack([r[c]["st_out"][:, 1] for c in range(NCORES)], axis=1)],
        axis=1).astype(np.float32)

    gam = np.asarray(bn_gamma, np.float32).reshape(C, 1)
    bet = np.asarray(bn_beta, np.float32).reshape(C, 1)
    in2 = [{"y_in": r[c]["y_out"], "allst": allst, "gam": gam, "bet": bet}
           for c in range(NCORES)]
    res2 = run_bass_kernel_spmd(nc_bn, in2, core_ids=list(range(NCORES)))

    # ---- assemble full outputs ----
    out = np.empty((B, C, N), np.float32)
    score_s = np.empty((B, N, N), np.float32)
    score_c = np.empty((B, N, N), np.float32)
    for core in range(NCORES):
        b, q = divmod(core, 4)
        sl = slice(q * NSL, (q + 1) * NSL)
        out[b][:, sl] = res2.results[core]["o_out"]
        score_s[b][sl, :] = r[core]["sco_s"]
        score_c[b][sl, :] = r[core]["sco_c"]
    return (out.reshape(B, C, H, W), score_s, score_c)
